# revision 16
# baseline (speedup 1.0000x reference)
"""Trainium2 Bass kernel for the CWLNFace margin-softmax loss head.

Reference computation (B=512, EMB=512, C=70722):
    kernel_norm = kernel / ||kernel||_col            # l2-normalize columns
    cosine      = clip(emb @ kernel_norm, -1+eps, 1-eps)
    out         = S * cos(clip(acos(cosine) - onehot*M*ms, eps, pi-eps))
                  - S * onehot*(M + M*ms)
For every non-label entry the acos/cos round-trip is the identity (the
theta clip never engages because |cosine| <= 1-eps keeps theta inside
[0.0447, pi-0.0447]), so the dense part is just  S * clip(cosine).  The
margin corrections touch exactly B=512 entries (one per row) and are
applied on the host from the device-computed clipped cosine values.

Device strategy (8 NeuronCores, classnum sharded):
    - Shard C across 8 cores (pad 70722 -> 8*8960 = 71680 with ones).
    - Each core computes  outT[c, b] = S * clip(dot(emb_b, k_c)/||k_c||)
      with the output transposed ([C_sh, B]) so the per-column norm scale
      is a per-partition scalar multiply.
    - Column norms via ACT Square (bf16) + PE matmul against a ones
      vector; main matmul in float32r (full PE rate at N=512).
Host reassembles, transposes, and patches the 512 label entries.
"""

import math
import numpy as np

B = 512
EMB = 512
C = 70722
NCORES = 8
CSH = 8960          # per-core padded classnum shard
NT = CSH // 128     # 70 C-tiles of 128 columns
S = 64.0
EPS = 1e-3
MARGIN = 0.4
H = 0.333
CLIP_HI = S * (1.0 - EPS)

_CACHE = {}


def _build_nc(reps=1):
    from contextlib import ExitStack

    from concourse import bacc, mybir, tile

    f32 = mybir.dt.float32
    f32r = mybir.dt.float32r
    bf16 = mybir.dt.bfloat16
    AF = mybir.ActivationFunctionType
    OP = mybir.AluOpType

    nc = bacc.Bacc(
        "TRN2",
        target_bir_lowering=False,
        debug=False,
        enable_asserts=False,
    )

    embT = nc.dram_tensor("embT", [EMB, B], f32, kind="ExternalInput").ap()
    # Host pre-tiles the shard so each C-tile is one contiguous 256 KiB
    # block: [tile, partition(EMB%128), chunk(EMB//128), col].
    ksh = nc.dram_tensor("ksh", [NT, 128, 4, 128], f32, kind="ExternalInput").ap()
    out = nc.dram_tensor("out", [CSH, B], f32, kind="ExternalOutput").ap()

    with tile.TileContext(nc) as tc, ExitStack() as ctx:
        singles = ctx.enter_context(tc.tile_pool(name="singles", bufs=1))
        kpool = ctx.enter_context(tc.tile_pool(name="k", bufs=6))
        kbpool = ctx.enter_context(tc.tile_pool(name="kb", bufs=6))
        sqpool = ctx.enter_context(tc.tile_pool(name="sq", bufs=4))
        opool = ctx.enter_context(tc.tile_pool(name="o", bufs=6))
        scpool = ctx.enter_context(tc.tile_pool(name="sc", bufs=8))
        pcpool = ctx.enter_context(tc.tile_pool(name="pc", bufs=4, space="PSUM"))
        pnpool = ctx.enter_context(tc.tile_pool(name="pn", bufs=4, space="PSUM"))

        # Embeddings^T resident in SBUF: [128, chunk, B], chunk = EMB/128.
        emb_f32 = singles.tile([128, 4, B], f32)
        nc.sync.dma_start(
            out=emb_f32[:], in_=embT.rearrange("(c p) b -> p c b", p=128)
        )
        emb_sb = singles.tile([128, 4, B], bf16)
        nc.vector.tensor_copy(emb_sb[:], emb_f32[:])
        ones_sb = singles.tile([128, 1], bf16)
        nc.vector.memset(ones_sb[:], 1.0)

        for t in [t for _ in range(reps) for t in range(NT)]:
            # Load one C-tile of the kernel shard: [128, chunk, 128].
            k_t = kpool.tile([128, 4, 128], f32)
            nc.sync.dma_start(out=k_t[:], in_=ksh[t])

            # bf16 copy of the tile (GpSimd, off the DVE/ACT critical path)
            kb_t = kbpool.tile([128, 4, 128], bf16)
            nc.gpsimd.tensor_copy(kb_t[:], k_t[:])

            # Column sum-of-squares via PE: normsq[c] = sum_k sq[k, c].
            sq_t = sqpool.tile([128, 4, 128], bf16)
            nc.vector.tensor_mul(sq_t[:], kb_t[:], kb_t[:])
            pn = pnpool.tile([128, 1], f32)
            for c in range(4):
                nc.tensor.matmul(
                    pn[:],
                    lhsT=sq_t[:, c, :],
                    rhs=ones_sb[:],
                    start=(c == 0),
                    stop=(c == 3),
                )
            # scale = S / sqrt(normsq)
            r_t = scpool.tile([128, 1], f32)
            nc.vector.reciprocal(r_t[:], pn[:])
            sc_t = scpool.tile([128, 1], f32)
            nc.scalar.activation(sc_t[:], r_t[:], AF.Sqrt, scale=S * S)

            # Main matmul: cosT_tile = ksh_tile^T @ embT  ([128 C, 512 B]).
            pc = pcpool.tile([128, B], f32)
            for c in range(4):
                nc.tensor.matmul(
                    pc[:],
                    lhsT=kb_t[:, c, :],
                    rhs=emb_sb[:, c, :],
                    start=(c == 0),
                    stop=(c == 3),
                )

            # out = clip(pc * scale, -CLIP_HI, CLIP_HI): ACT does the
            # per-partition scale copy (PSUM->SBUF), DVE one fused clip.
            o_t = opool.tile([128, B], f32)
            nc.scalar.activation(o_t[:], pc[:], AF.Copy, scale=sc_t[:])
            nc.vector.tensor_scalar(
                o_t[:], o_t[:], CLIP_HI, -CLIP_HI, OP.min, OP.max
            )
            nc.sync.dma_start(out=out[t * 128 : (t + 1) * 128, :], in_=o_t[:])

    nc.compile()
    return nc


def _get_nc():
    if "nc" not in _CACHE:
        _CACHE["nc"] = _build_nc()
    return _CACHE["nc"]


def make_shards(kfull):
    """Split kernel [EMB, C] into per-core tile-major shards
    [NT, 128, 4, 128] (each C-tile contiguous)."""
    shards = []
    for i in range(NCORES):
        lo, hi = i * CSH, (i + 1) * CSH
        if hi <= C:
            shard = kfull[:, lo:hi]
        else:
            shard = np.ones((EMB, CSH), dtype=np.float32)
            shard[:, : C - lo] = kfull[:, lo:C]
        # rows = (chunk, p), cols = (tile, w)  ->  [tile, p, chunk, w]
        tiled = shard.reshape(4, 128, NT, 128).transpose(2, 1, 0, 3)
        shards.append(np.ascontiguousarray(tiled))
    return shards


def run_device(embbedings, kernel, trace=False):
    """Run the sharded device kernel. Returns (outT [C,B] float32, results)."""
    from concourse.bass_utils import run_bass_kernel_spmd

    nc = _get_nc()

    embT = np.ascontiguousarray(np.asarray(embbedings, dtype=np.float32).T)
    kfull = np.asarray(kernel, dtype=np.float32)

    in_maps = [
        {"embT": embT, "ksh": shard} for shard in make_shards(kfull)
    ]

    res = run_bass_kernel_spmd(nc, in_maps, core_ids=list(range(NCORES)), trace=trace)
    outT = np.concatenate([r["out"] for r in res.results], axis=0)[:C]  # [C, B]
    return outT, res


def kernel(embbedings, norms, label, class_sample_num_, kernel):
    outT, _ = run_device(embbedings, kernel)

    # ---- host margin fix-up (touches exactly B entries) ----
    norms = np.asarray(norms, dtype=np.float32)
    csn = np.asarray(class_sample_num_, dtype=np.float32)
    lab = np.asarray(label).astype(np.int64)

    safe = np.clip(norms, 0.001, 100.0)
    safe = safe / (csn[:, None] + 0.001)
    safe = np.clip(safe, 0.001, 100.0).astype(np.float32)
    mean = safe.mean(dtype=np.float64)
    std = safe.std(ddof=1, dtype=np.float64)
    ms = np.clip((safe.astype(np.float64) - mean) / (std + EPS) * H, -1.0, 1.0)[:, 0]

    rows = np.arange(B)
    c0 = outT[lab, rows].astype(np.float64) / S  # clip(cosine) at label cols
    theta = np.arccos(c0) - MARGIN * ms
    theta = np.clip(theta, EPS, math.pi - EPS)
    val = (np.cos(theta) - (MARGIN + MARGIN * ms)) * S
    outT[lab, rows] = val.astype(np.float32)

    return np.ascontiguousarray(outT.T)


# revision 17
# speedup vs baseline: 1.3234x; 1.3234x over previous
"""Trainium2 Bass kernel for the CWLNFace margin-softmax loss head.

Reference computation (B=512, EMB=512, C=70722):
    kernel_norm = kernel / ||kernel||_col            # l2-normalize columns
    cosine      = clip(emb @ kernel_norm, -1+eps, 1-eps)
    out         = S * cos(clip(acos(cosine) - onehot*M*ms, eps, pi-eps))
                  - S * onehot*(M + M*ms)
For every non-label entry the acos/cos round-trip is the identity (the
theta clip never engages because |cosine| <= 1-eps keeps theta inside
[0.0447, pi-0.0447]), so the dense part is just  S * clip(cosine).  The
margin corrections touch exactly B=512 entries (one per row) and are
applied on the host from the device-computed clipped cosine values.

Device strategy (8 NeuronCores, classnum sharded):
    - Shard C across 8 cores (pad 70722 -> 8*8960 = 71680 with ones).
    - Each core computes  outT[c, b] = S * clip(dot(emb_b, k_c)/||k_c||)
      with the output transposed ([C_sh, B]) so the per-column norm scale
      is a per-partition scalar multiply.
    - Column norms via ACT Square (bf16) + PE matmul against a ones
      vector; main matmul in float32r (full PE rate at N=512).
Host reassembles, transposes, and patches the 512 label entries.
"""

import math
import numpy as np

B = 512
EMB = 512
C = 70722
NCORES = 8
CSH = 8960          # per-core padded classnum shard
NT = CSH // 128     # 70 C-tiles of 128 columns
S = 64.0
EPS = 1e-3
MARGIN = 0.4
H = 0.333
CLIP_HI = S * (1.0 - EPS)

_CACHE = {}


def _build_nc(reps=1):
    from contextlib import ExitStack

    from concourse import bacc, mybir, tile

    f32 = mybir.dt.float32
    f32r = mybir.dt.float32r
    bf16 = mybir.dt.bfloat16
    AF = mybir.ActivationFunctionType
    OP = mybir.AluOpType

    nc = bacc.Bacc(
        "TRN2",
        target_bir_lowering=False,
        debug=False,
        enable_asserts=False,
    )

    embT = nc.dram_tensor("embT", [EMB, B], f32, kind="ExternalInput").ap()
    # Host pre-tiles the shard so each C-tile is one contiguous 256 KiB
    # block: [tile, partition(EMB%128), chunk(EMB//128), col].
    ksh = nc.dram_tensor("ksh", [NT, 128, 4, 128], f32, kind="ExternalInput").ap()
    out = nc.dram_tensor("out", [CSH, B], f32, kind="ExternalOutput").ap()

    with tile.TileContext(nc) as tc, ExitStack() as ctx:
        singles = ctx.enter_context(tc.tile_pool(name="singles", bufs=1))
        kpool = ctx.enter_context(tc.tile_pool(name="k", bufs=6))
        kbpool = ctx.enter_context(tc.tile_pool(name="kb", bufs=6))
        sqpool = ctx.enter_context(tc.tile_pool(name="sq", bufs=4))
        opool = ctx.enter_context(tc.tile_pool(name="o", bufs=6))
        scpool = ctx.enter_context(tc.tile_pool(name="sc", bufs=8))
        pcpool = ctx.enter_context(tc.tile_pool(name="pc", bufs=4, space="PSUM"))
        pnpool = ctx.enter_context(tc.tile_pool(name="pn", bufs=4, space="PSUM"))

        # Embeddings^T resident in SBUF: [128, chunk, B], chunk = EMB/128.
        emb_f32 = singles.tile([128, 4, B], f32)
        nc.sync.dma_start(
            out=emb_f32[:], in_=embT.rearrange("(c p) b -> p c b", p=128)
        )
        emb_sb = singles.tile([128, 4, B], bf16)
        nc.vector.tensor_copy(emb_sb[:], emb_f32[:])
        ones_sb = singles.tile([128, 1], bf16)
        nc.vector.memset(ones_sb[:], 1.0)

        for t in [t for _ in range(reps) for t in range(NT)]:
            # Load one C-tile of the kernel shard: [128, chunk, 128].
            # Input loads on the ACT HWDGE ring, output stores on the SP
            # ring: one sequencer issuing both would execute out[t]'s data
            # wait inline and block the issue of in[t+1] behind the whole
            # compute chain, capping the pipeline at ~2 tiles.
            k_t = kpool.tile([128, 4, 128], f32)
            nc.scalar.dma_start(out=k_t[:], in_=ksh[t])

            # bf16 copy of the tile (GpSimd, off the DVE/ACT critical path)
            kb_t = kbpool.tile([128, 4, 128], bf16)
            nc.gpsimd.tensor_copy(kb_t[:], k_t[:])

            # Column sum-of-squares via PE: normsq[c] = sum_k sq[k, c].
            sq_t = sqpool.tile([128, 4, 128], bf16)
            nc.vector.tensor_mul(sq_t[:], kb_t[:], kb_t[:])
            pn = pnpool.tile([128, 1], f32)
            for c in range(4):
                nc.tensor.matmul(
                    pn[:],
                    lhsT=sq_t[:, c, :],
                    rhs=ones_sb[:],
                    start=(c == 0),
                    stop=(c == 3),
                )
            # scale = S / sqrt(normsq)
            r_t = scpool.tile([128, 1], f32)
            nc.vector.reciprocal(r_t[:], pn[:])
            sc_t = scpool.tile([128, 1], f32)
            nc.scalar.activation(sc_t[:], r_t[:], AF.Sqrt, scale=S * S)

            # Main matmul: cosT_tile = ksh_tile^T @ embT  ([128 C, 512 B]).
            pc = pcpool.tile([128, B], f32)
            for c in range(4):
                nc.tensor.matmul(
                    pc[:],
                    lhsT=kb_t[:, c, :],
                    rhs=emb_sb[:, c, :],
                    start=(c == 0),
                    stop=(c == 3),
                )

            # out = clip(pc * scale, -CLIP_HI, CLIP_HI): ACT does the
            # per-partition scale copy (PSUM->SBUF), DVE one fused clip.
            o_t = opool.tile([128, B], f32)
            nc.scalar.activation(o_t[:], pc[:], AF.Copy, scale=sc_t[:])
            nc.vector.tensor_scalar(
                o_t[:], o_t[:], CLIP_HI, -CLIP_HI, OP.min, OP.max
            )
            nc.sync.dma_start(out=out[t * 128 : (t + 1) * 128, :], in_=o_t[:])

    nc.compile()
    return nc


def _get_nc():
    if "nc" not in _CACHE:
        _CACHE["nc"] = _build_nc()
    return _CACHE["nc"]


def make_shards(kfull):
    """Split kernel [EMB, C] into per-core tile-major shards
    [NT, 128, 4, 128] (each C-tile contiguous)."""
    shards = []
    for i in range(NCORES):
        lo, hi = i * CSH, (i + 1) * CSH
        if hi <= C:
            shard = kfull[:, lo:hi]
        else:
            shard = np.ones((EMB, CSH), dtype=np.float32)
            shard[:, : C - lo] = kfull[:, lo:C]
        # rows = (chunk, p), cols = (tile, w)  ->  [tile, p, chunk, w]
        tiled = shard.reshape(4, 128, NT, 128).transpose(2, 1, 0, 3)
        shards.append(np.ascontiguousarray(tiled))
    return shards


def run_device(embbedings, kernel, trace=False):
    """Run the sharded device kernel. Returns (outT [C,B] float32, results)."""
    from concourse.bass_utils import run_bass_kernel_spmd

    nc = _get_nc()

    embT = np.ascontiguousarray(np.asarray(embbedings, dtype=np.float32).T)
    kfull = np.asarray(kernel, dtype=np.float32)

    in_maps = [
        {"embT": embT, "ksh": shard} for shard in make_shards(kfull)
    ]

    res = run_bass_kernel_spmd(nc, in_maps, core_ids=list(range(NCORES)), trace=trace)
    outT = np.concatenate([r["out"] for r in res.results], axis=0)[:C]  # [C, B]
    return outT, res


def kernel(embbedings, norms, label, class_sample_num_, kernel):
    outT, _ = run_device(embbedings, kernel)

    # ---- host margin fix-up (touches exactly B entries) ----
    norms = np.asarray(norms, dtype=np.float32)
    csn = np.asarray(class_sample_num_, dtype=np.float32)
    lab = np.asarray(label).astype(np.int64)

    safe = np.clip(norms, 0.001, 100.0)
    safe = safe / (csn[:, None] + 0.001)
    safe = np.clip(safe, 0.001, 100.0).astype(np.float32)
    mean = safe.mean(dtype=np.float64)
    std = safe.std(ddof=1, dtype=np.float64)
    ms = np.clip((safe.astype(np.float64) - mean) / (std + EPS) * H, -1.0, 1.0)[:, 0]

    rows = np.arange(B)
    c0 = outT[lab, rows].astype(np.float64) / S  # clip(cosine) at label cols
    theta = np.arccos(c0) - MARGIN * ms
    theta = np.clip(theta, EPS, math.pi - EPS)
    val = (np.cos(theta) - (MARGIN + MARGIN * ms)) * S
    outT[lab, rows] = val.astype(np.float32)

    return np.ascontiguousarray(outT.T)
